# revision 4
# baseline (speedup 1.0000x reference)
"""Trainium2 Bass kernel: weighted sliding-window min (STL 'Always' robustness).

out[n, w] = min_k( input[n, 4*w + k] * And_weight[0, k] ),  k in [0, 16)

Strategy (8 NeuronCores, data-parallel over batch N=1024 -> 128 rows/core):
  - Host: cast to bf16, deinterleave each row into 4 phase planes
    P_j[b] = x[4b + j]; prepend the 16 fp32 And_weights bit-packed into 32
    bf16 slots (read back on-device via bitcast APs, full fp32 precision).
  - Device: 16 products p_{o,j}[w] = P_j[w+o] * c[4o+j] with the window
    shift o folded into the product *read* offset, so every min-tree level
    is a dense shift-free tensor_tensor (bf16 2x mode, no halo logic).
    DVE computes the even-o products (4B-aligned reads, TS 4x mode);
    ScalarE (ACT) computes the odd-o products in the shadow of the tree.
    GPSIMD independently computes the last GPW windows end-to-end
    (tensor_scalar products + tensor_tensor min chains), removing that
    column range from the DVE/ACT critical path entirely.
  - Slot layout s(o, jh=j//2) = 4*(o%2) + 2*(o//2) + jh groups the tree so
    l1/L2 split into an even-o half (DVE-fed, runs early) and an odd-o
    half (ACT-fed, column-chunked x3 against ACT's last product pieces).
  - Output stored in 4 column chunks on the two HWDGE rings as each chunk
    completes, hiding the ~2us DMA completion latency.
"""

import os as _os

import numpy as np

# Problem geometry (hardcoded; harness calls kernel() with these shapes)
N, L = 1024, 8192
K, S = 16, 4
W = (L - K) // S + 1          # 2045 output windows per row
NCORES = 8
ROWS = N // NCORES            # 128 rows per core == SBUF partitions
B = L // S                    # 2048 blocks of 4 per row
FLAT = 32 + 4 * B             # 32 bf16 slots = 16 fp32 weights bit-packed

GPW = int(_os.environ.get("K_GPW", "317"))   # windows handled by GPSIMD
WD = W - GPW                                  # DVE/ACT windows [0, WD)
GPB = B - WD                                  # GP block range [WD, B)
CH0 = int(_os.environ.get("K_CH0", "512"))   # first chunk of plane 0
_CB = _os.environ.get("K_CB", "")             # chunk boundaries (w space)
if _CB:
    CBS = [int(v) for v in _CB.split(",")]
else:
    CBS = [WD // 2 if WD // 2 % 2 == 0 else WD // 2 + 1]
    CBS.append((CBS[0] + WD) // 2 // 2 * 2)
CHUNKS = [(a, b) for a, b in zip([0] + CBS, CBS + [WD])]
# extra products moved from DVE to ACT, e.g. "03" -> (o=0, j=3)
_AX = _os.environ.get("K_AX", "")
ACT_EXTRA = {(int(_AX[i]), int(_AX[i + 1])) for i in range(0, len(_AX), 2)}

_COMPILED = {}


def _build_bass():
    import concourse.bacc as bacc
    import concourse.mybir as mybir
    from concourse.tile import TileContext

    BF16 = mybir.dt.bfloat16
    F32 = mybir.dt.float32
    MIN = mybir.AluOpType.min

    nc = bacc.Bacc(enable_partition_id=False)
    x = nc.dram_tensor("x", [ROWS, FLAT], BF16, kind="ExternalInput")
    out = nc.dram_tensor("out", [ROWS, W], BF16, kind="ExternalOutput")

    # slot layout: even-o products in slots 0:4 (DVE-fed), odd-o in 4:8
    # (ACT-fed).  Within a half: [o_lo jh0, o_lo jh1, o_hi jh0, o_hi jh1],
    # so L2 (fold o -> o+2) is min(half[0:2], half[2:4]).
    def slot(o, jh):
        return 4 * (o % 2) + 2 * (o // 2) + jh

    with TileContext(nc) as tc:
        with (
            tc.tile_pool(name="wp", bufs=1) as wp,
            tc.tile_pool(name="xin", bufs=1) as xin,
            tc.tile_pool(name="pa", bufs=1) as pa,
            tc.tile_pool(name="qq", bufs=1) as qq,
            tc.tile_pool(name="uu", bufs=1) as uu,
            tc.tile_pool(name="oo", bufs=1) as oo,
            tc.tile_pool(name="gg", bufs=1) as gg,
        ):
            # Dummy first Activation so Bacc hoists the ACT table load to
            # the top of the Scalar queue.
            dummy = wp.tile([ROWS, 1], F32)
            nc.scalar.memzero(dummy[:, :])

            # Input DMAs in consumption order on the Sync HWDGE ring.
            xw0 = xin.tile([ROWS, 32 + CH0], BF16, tag="xw0")
            nc.sync.dma_start(out=xw0[:, :], in_=x[:, 0 : 32 + CH0])
            x0b = xin.tile([ROWS, B - CH0], BF16, tag="x0b")
            nc.sync.dma_start(out=x0b[:, :], in_=x[:, 32 + CH0 : 32 + B])
            pl = [None]
            for j in (1, 2, 3):
                t = xin.tile([ROWS, B], BF16, tag=f"p{j}")
                nc.sync.dma_start(out=t[:, :], in_=x[:, 32 + j * B : 32 + (j + 1) * B])
                pl.append(t)

            def plane(j, lo, hi):
                """AP for P_j[lo:hi] (lo/hi in block units)."""
                if j == 0:
                    if hi <= CH0:
                        return [xw0[:, 32 + lo : 32 + hi]]
                    if lo >= CH0:
                        return [x0b[:, lo - CH0 : hi - CH0]]
                    return [xw0[:, 32 + lo : 32 + CH0], x0b[:, 0 : hi - CH0]]
                return [pl[j][:, lo:hi]]

            def sc(k):
                return xw0[:, 2 * k : 2 * k + 2].bitcast(F32)

            A = pa.tile([ROWS, 8, WD], BF16, tag="A")
            Bb = pa.tile([ROWS, 8, WD], BF16, tag="B")

            def emit_mul(o, j, eng):
                dst = A if (j % 2 == 0) else Bb
                s = slot(o, j // 2)
                # product reads P_j[w+o] for w in [0, WD)
                pieces = []
                if j == 0:
                    if o < CH0:
                        pieces.append((0, CH0 - o))
                    pieces.append((max(0, CH0 - o), WD))
                else:
                    pieces.append((0, WD))
                for lo, hi in pieces:
                    for ap in plane(j, lo + o, hi + o):
                        w_ = ap.shape[-1]
                        if eng == "act":
                            nc.scalar.mul(
                                out=dst[:, s, lo : lo + w_], in_=ap, mul=sc(4 * o + j)
                            )
                        else:
                            nc.vector.tensor_scalar_mul(
                                out=dst[:, s, lo : lo + w_], in0=ap, scalar1=sc(4 * o + j)
                            )
                        lo += w_

            # ---- products ----
            # DVE: even o (4B-aligned reads -> TS 4x); ACT: odd o.
            dve_prods = [(o, j) for j in range(4) for o in (0, 2)]
            dve_prods = [p for p in dve_prods if p not in ACT_EXTRA]
            act_head = [(1, 0), (3, 0), (1, 1), (3, 1), (1, 2), (1, 3)]
            act_head += sorted(ACT_EXTRA, key=lambda p: p[1])
            act_tail = [(3, 2), (3, 3)]  # chunked, interleaved per chunk

            for o, j in dve_prods[:4]:
                emit_mul(o, j, "dve")

            # GPSIMD: full subtree for windows [WD, W).  m_o[i] =
            # min_j P_j[WD+i]*c[4o+j], i in [0, GPB); then fold o with
            # shifts and write ot[WD:W].
            gm = gg.tile([ROWS, 4, GPB], BF16, tag="gm")
            gt = gg.tile([ROWS, GPB], BF16, tag="gt")

            def gp_round(j):
                for o in range(4):
                    src = plane(j, WD, B)
                    assert len(src) == 1
                    if j == 0:
                        nc.gpsimd.tensor_scalar_mul(
                            out=gm[:, o, :], in0=src[0], scalar1=sc(4 * o + j)
                        )
                    else:
                        nc.gpsimd.tensor_scalar_mul(
                            out=gt[:, :], in0=src[0], scalar1=sc(4 * o + j)
                        )
                        nc.gpsimd.tensor_tensor(
                            out=gm[:, o, :], in0=gt[:, :], in1=gm[:, o, :], op=MIN
                        )

            if GPW > 0:
                gp_round(0)

            for o, j in dve_prods[4:]:
                emit_mul(o, j, "dve")
            for o, j in act_head:
                emit_mul(o, j, "act")
            if GPW > 0:
                for j in (1, 2, 3):
                    gp_round(j)

            Q = qq.tile([ROWS, 8, WD], BF16, tag="Q")
            U = uu.tile([ROWS, 4, WD], BF16, tag="U")  # [0:2]=even-o, [2:4]=odd-o
            V = uu.tile([ROWS, 2, WD], BF16, tag="V")
            ot = oo.tile([ROWS, W], BF16, tag="ot")

            # ---- even-o half: fully DVE-fed, runs during ACT's products ----
            nc.vector.tensor_tensor(
                out=Q[:, 0:4, :], in0=A[:, 0:4, :], in1=Bb[:, 0:4, :], op=MIN
            )
            nc.vector.tensor_tensor(
                out=U[:, 0:2, :], in0=Q[:, 0:2, :], in1=Q[:, 2:4, :], op=MIN
            )

            # ---- GP folds + store (independent of DVE/ACT) ----
            if GPW > 0:
                g2 = gg.tile([ROWS, 2, GPW + 1], BF16, tag="g2")
                nc.gpsimd.tensor_tensor(
                    out=g2[:, 0, :], in0=gm[:, 0, 0 : GPW + 1],
                    in1=gm[:, 2, 2 : GPW + 3], op=MIN,
                )
                nc.gpsimd.tensor_tensor(
                    out=g2[:, 1, 0:GPW], in0=gm[:, 1, 1 : GPW + 1],
                    in1=gm[:, 3, 3 : GPW + 3], op=MIN,
                )
                nc.gpsimd.tensor_tensor(
                    out=ot[:, WD:W], in0=g2[:, 0, 0:GPW], in1=g2[:, 1, 0:GPW], op=MIN
                )

            # ---- odd-o half: chunked against ACT's last product pieces ----
            for ci, (c0, c1) in enumerate(CHUNKS):
                for o, j in act_tail:
                    dst = A if (j % 2 == 0) else Bb
                    s = slot(o, j // 2)
                    ap = plane(j, c0 + o, c1 + o)
                    assert len(ap) == 1
                    nc.scalar.mul(out=dst[:, s, c0:c1], in_=ap[0], mul=sc(4 * o + j))
                nc.vector.tensor_tensor(
                    out=Q[:, 4:8, c0:c1], in0=A[:, 4:8, c0:c1], in1=Bb[:, 4:8, c0:c1],
                    op=MIN,
                )
                nc.vector.tensor_tensor(
                    out=U[:, 2:4, c0:c1], in0=Q[:, 4:6, c0:c1], in1=Q[:, 6:8, c0:c1],
                    op=MIN,
                )
                nc.vector.tensor_tensor(
                    out=V[:, :, c0:c1], in0=U[:, 0:2, c0:c1], in1=U[:, 2:4, c0:c1],
                    op=MIN,
                )
                nc.vector.tensor_tensor(
                    out=ot[:, c0:c1], in0=V[:, 0, c0:c1], in1=V[:, 1, c0:c1], op=MIN
                )
                if ci == len(CHUNKS) - 1:
                    nc.scalar.dma_start(out=out[:, c0:c1], in_=ot[:, c0:c1])
                else:
                    nc.sync.dma_start(out=out[:, c0:c1], in_=ot[:, c0:c1])
            if GPW > 0:
                nc.sync.dma_start(out=out[:, WD:W], in_=ot[:, WD:W])
    nc.finalize()
    return nc


def _host_prep(input_f32, And_weight):
    """Shard + relayout host-side. Returns in_maps for the 8 cores."""
    import ml_dtypes

    xb = np.asarray(input_f32, dtype=np.float32).astype(ml_dtypes.bfloat16)
    # [N, L] -> [N, B, 4] -> [N, 4, B] phase planes
    planes = np.ascontiguousarray(xb.reshape(N, B, S).transpose(0, 2, 1))

    flat = np.zeros((N, FLAT), dtype=ml_dtypes.bfloat16)
    wbits = (
        np.asarray(And_weight, dtype=np.float32)
        .reshape(K)
        .view(np.uint16)
        .view(ml_dtypes.bfloat16)
    )
    flat[:, 0:32] = wbits[None, :]
    flat[:, 32:] = planes.reshape(N, 4 * B)

    in_maps = []
    for c in range(NCORES):
        in_maps.append({"x": np.ascontiguousarray(flat[c * ROWS : (c + 1) * ROWS])})
    return in_maps


def _get_nc():
    if "nc" not in _COMPILED:
        _COMPILED["nc"] = _build_bass()
    return _COMPILED["nc"]


def _run(in_maps, trace=False, **kw):
    from concourse.bass_utils import run_bass_kernel_spmd

    nc = _get_nc()
    res = run_bass_kernel_spmd(
        nc, in_maps, core_ids=list(range(NCORES)), trace=trace, **kw
    )
    return res


def kernel(input, And_weight):
    in_maps = _host_prep(input, And_weight)
    res = _run(in_maps, trace=False)
    out = np.concatenate([res.results[c]["out"] for c in range(NCORES)], axis=0)
    return out.astype(np.float32)
